# revision 49
# baseline (speedup 1.0000x reference)
"""Trainium2 Bass kernel for nn_AttentionSelector (segment softmax attention).

Math shortcut: logits = segment_sum(w * repre) @ relation_mat.T + bias is
linear in repre, so with P = repre @ relation_mat.T ([N,53]) the whole
computation lives in 53-dim space:
    x_i   = P[i, labels[i]]          (rel logit per instance)
    e_i   = exp(x_i)                 (logits are ~N(0, 0.026^2): no max needed)
    out_b = (sum_{i in b} e_i P[i,:]) / (sum_{i in b} e_i) + bias

Device pipeline (per core, bags sharded 3125/core, rows padded to Rpad):
  A:  stream X^T in bf16 (the HBM roofline) as [128, 6*1024]-blocks
      (128-partition, 12KB contiguous lines -> ~340GB/s vs 135 for 2KB
      packets); 6 accumulating matmuls (D padded 690->768=6*128) per
      512-col half -> P^T in PSUM; scalar-engine copy to bf16 SBUF.
  T:  PE-transposes P^T -> row-major 128-row chunks (4 chunks per PSUM
      tile); per chunk: fused DVE multiply-reduce against a host-built
      one-hot label mask extracts x; scalar engine exp writes e directly
      into the P_aug e-column; pool engine scales P by e into P_aug and
      builds the one-hot slot matrix H0 from host-built seg ids.
  C:  per chunk one matmul att_un[s,0:54] = sum_i H0[i,s]*[eP | e][i,:]
      accumulated 9 chunks per PSUM bank; DVE copies groups to SBUF.
  Host compacts the <=2 (chunk,slot) partials per bag, divides by den,
  adds bias. All DMAs are large contiguous-per-partition transfers
  (no 4-byte scatter packets anywhere).
"""
import math
import os
import sys

for _p in ("/opt/trn_rl_repo", "/opt/trn_rl_repo/concourse", "/opt/pypackages"):
    if _p not in sys.path:
        sys.path.insert(0, _p)

import numpy as np
import ml_dtypes

BF16 = ml_dtypes.bfloat16
FP8 = ml_dtypes.float8_e4m3fn

N_TOTAL = 200000
NUM_BAGS = 25000
DIM = 690
DPAD = 768         # 6 * 128
KCH = 128
NK = 6
REL = 53
AUG = REL + 1      # 53 P-columns + e column
GRP = 9            # attention chunks accumulated per PSUM bank (9*54=486 f32)
NCORES = 8

LAST_RESULTS = None
_PROGRAM_CACHE = {}


def _build_program(Rpad, debug_out=False, stages="ATC", scalar_dma=True):
    from concourse import bacc, mybir
    import concourse.tile as tile
    from concourse.masks import make_identity

    f32 = mybir.dt.float32
    bf16 = mybir.dt.bfloat16
    fp8 = mybir.dt.float8e4
    Alu = mybir.AluOpType
    Act = mybir.ActivationFunctionType
    NJ = Rpad // 1024
    NCH = Rpad // 128
    T = 2 * NJ                      # number of 512-col halves

    nc = bacc.Bacc("TRN2", target_bir_lowering=False, debug=False,
                   enable_asserts=False)

    with tile.TileContext(nc) as tc:
        with tc.tile_pool(name="dram", bufs=1, space="DRAM") as dram, \
             tc.tile_pool(name="consts", bufs=1) as consts, \
             tc.tile_pool(name="xt", bufs=4) as xtp, \
             tc.tile_pool(name="junk", bufs=3) as junkp, \
             tc.tile_pool(name="big", bufs=1) as bigp, \
             tc.tile_pool(name="pt_ps", bufs=2, space="PSUM") as ptps, \
             tc.tile_pool(name="tr_ps", bufs=2, space="PSUM") as trps, \
             tc.tile_pool(name="x_ps", bufs=2, space="PSUM") as xps, \
             tc.tile_pool(name="c_ps", bufs=2, space="PSUM") as cps:

            # merged per-block stream: [xt bf16 | h0 fp8 | ot fp8] with
            # uniform 14336-byte partition lines (one DMA per block)
            BCOLS = NK * 1024 + 512 + 1024      # in bf16 columns
            H0OFF = NK * 1024                   # h0: 512 bf16 cols = 1024 fp8
            OTOFF = NK * 1024 + 512             # ot: 1024 bf16 cols
            xt_d = dram.tile([NJ, 128, BCOLS], bf16, kind="ExternalInput",
                             name="xtb", uniquify=False)
            wm_d = dram.tile([128, NK, REL], bf16, kind="ExternalInput",
                             name="wmb", uniquify=False)
            att_d = dram.tile([128, NCH * AUG], bf16, kind="ExternalOutput",
                              name="attstage", uniquify=False)
            _dbg = dict(kind="ExternalOutput", uniquify=False) if debug_out \
                else {}
            e_d = dram.tile([128, NCH], f32, name="edbg", **_dbg)

            # constants
            wm_sb = consts.tile([128, NK, REL], bf16, name="wm_sb", tag="wm_sb")
            nc.sync.dma_start(wm_sb[:], wm_d[:])
            identb = consts.tile([128, 128], bf16, name="identb", tag="identb")
            make_identity(nc, identb[:])
            onesb = consts.tile([REL, 1], bf16, name="onesb", tag="onesb")
            nc.vector.memset(onesb[:], 1.0)

            P_all = bigp.tile([128, NCH * AUG], bf16, name="P_all",
                              tag="P_all")
            attst = bigp.tile([128, NCH * AUG], bf16, name="attst",
                              tag="attst")
            if "C" not in stages:
                nc.vector.memset(attst[:], 0.0)
            e_all = bigp.tile([128, NCH], f32, name="e_all", tag="e_all")
            # static P^T staging tiles with a permanent ones-row: transposing
            # [54,128] blocks yields [P | 1] slots, so den comes for free
            pt_sbs = []
            for i in range(3):
                t_ = consts.tile([AUG, 512], bf16, name=f"pt_sb{i}",
                                 tag=f"pt_sb{i}")
                nc.vector.memset(t_[:], 1.0)
                pt_sbs.append(t_)

            xt_tiles = {}
            pt_tiles = {}
            junk_tiles = {}
            tr_tiles = {}
            cgrp = {"tile": None, "base": 0, "cnt": 0}

            out_state = {"done": 0}

            def flush_cgrp(final=False):
                t_, base, cnt = cgrp["tile"], cgrp["base"], cgrp["cnt"]
                if t_ is not None and cnt > 0:
                    nc.vector.tensor_copy(
                        attst[:, AUG * base:AUG * (base + cnt)],
                        t_[:, :AUG * cnt])
                    cgrp["tile"] = None
                    cgrp["cnt"] = 0
                # stream finished attst ranges out instead of one tail DMA
                hi = base + cnt if t_ is not None else out_state["done"]
                if final:
                    hi = NCH
                if hi - out_state["done"] >= 45 or \
                        (final and hi > out_state["done"]):
                    lo = out_state["done"]
                    nc.scalar.dma_start(att_d[:, AUG * lo:AUG * hi],
                                      attst[:, AUG * lo:AUG * hi])
                    out_state["done"] = hi

            for t in range(T + 2):
                # ---- stage A: matmuls for half t ----
                if t < T:
                    j, h = divmod(t, 2)
                    if h == 0:
                        xt = xtp.tile([128, BCOLS], bf16, name="xt",
                                      tag="xt")
                        nc.sync.dma_start(xt[:], xt_d[j])
                        xt_tiles[j] = xt
                    xt = xt_tiles[j]
                    pt_ps = ptps.tile([REL, 512], f32, space="PSUM",
                                      name="pt_ps", tag="pt_ps")
                    for k in range(NK):
                        nc.tensor.matmul(
                            pt_ps[:], wm_sb[:, k, :],
                            xt[:, 1024 * k + 512 * h:1024 * k + 512 * (h + 1)],
                            start=(k == 0), stop=(k == NK - 1))
                    pt_sb = pt_sbs[t % 3]
                    nc.scalar.activation(pt_sb[:REL, :], pt_ps[:], Act.Copy)
                    pt_tiles[t] = pt_sb
                    junk = junkp.tile([REL, 512], bf16, name="junk",
                                      tag="junk")
                    nc.vector.tensor_tensor(
                        out=junk[:], in0=pt_sb[:REL, :],
                        in1=xt[:REL, OTOFF + 512 * h:OTOFF + 512 * (h + 1)],
                        op=Alu.mult)
                    junk_tiles[t] = junk

                # ---- stage T: x / transpose / e-scaled P_aug for half t-1 --
                u = t - 1
                if "T" in stages and 0 <= u < T:
                    pt_sb = pt_tiles.pop(u)
                    junk = junk_tiles.pop(u)
                    c0 = 4 * u
                    tr4 = trps.tile([128, 4 * AUG], bf16, space="PSUM",
                                    name="tr4", tag="tr4")
                    for q in range(4):
                        nc.tensor.transpose(
                            tr4[:, AUG * q:AUG * (q + 1)],
                            pt_sb[:, 128 * q:128 * (q + 1)],
                            identb[:AUG, :AUG])
                    tr_tiles[u] = tr4
                    x4 = xps.tile([128, 4], f32, space="PSUM",
                                  name="x4", tag="x4")
                    for q in range(4):
                        nc.tensor.matmul(
                            x4[:, q:q + 1], junk[:, 128 * q:128 * (q + 1)],
                            onesb[:], start=True, stop=True)
                    nc.scalar.activation(e_all[:, c0:c0 + 4], x4[:], Act.Exp)
                    # PSUM -> SBUF with per-row e scaling; the ones-row of
                    # pt_sb becomes the e column (the den accumulator).
                    # Split across scalar and vector engines.
                    for q in range(4):
                        c = c0 + q
                        dst = P_all[:, AUG * c:AUG * (c + 1)]
                        src = tr4[:, AUG * q:AUG * (q + 1)]
                        if q < 1:
                            nc.scalar.activation(dst, src, Act.Copy,
                                                 scale=e_all[:, c:c + 1])
                        else:
                            nc.vector.tensor_scalar(
                                dst, src, e_all[:, c:c + 1], None, Alu.mult)

                # ---- stage C: attention matmuls for half t-2 ----
                v = t - 2
                if "C" in stages and 0 <= v < T:
                    tr_tiles.pop(v, None)
                    for q in range(4):
                        c = 4 * v + q
                        if cgrp["tile"] is None:
                            cgrp["tile"] = cps.tile(
                                [128, GRP * AUG], f32, space="PSUM",
                                name="cacc", tag="cacc")
                            cgrp["base"] = c
                        off = AUG * (c - cgrp["base"])
                        cj, cq = divmod(c, 8)
                        h0v = xt_tiles[cj][:, H0OFF + 64 * cq:
                                           H0OFF + 64 * (cq + 1)].bitcast(fp8)
                        nc.tensor.matmul(
                            cgrp["tile"][:, off:off + AUG], h0v,
                            P_all[:, AUG * c:AUG * (c + 1)],
                            start=True, stop=True)
                        cgrp["cnt"] += 1
                        if cgrp["cnt"] == GRP:
                            flush_cgrp()
            flush_cgrp(final=True)
            if debug_out:
                nc.sync.dma_start(e_d[:], e_all[:])

    nc.compile()
    return nc


def _prep(repre, relation_mat, bias, scope, labels, ncores):
    repre = np.asarray(repre, dtype=np.float32)
    relmat = np.asarray(relation_mat, dtype=np.float32)
    bias_np = np.asarray(bias, dtype=np.float32)
    scope = np.asarray(scope).astype(np.int64)
    labels_np = np.asarray(labels).astype(np.int64)
    n, d = repre.shape
    nbags = scope.shape[0]
    assert d == DIM and nbags % ncores == 0
    bpc = nbags // ncores
    starts, ends = scope[:, 0], scope[:, 1]
    lens = ends - starts
    core_r0 = starts[np.arange(ncores) * bpc]
    core_r1 = ends[np.arange(ncores) * bpc + bpc - 1]
    rows = core_r1 - core_r0
    Rpad = int(1024 * math.ceil(int(rows.max()) / 1024))
    NCH = Rpad // 128
    NJ = Rpad // 1024
    assert int(lens.max()) <= 128, "bag too large for this kernel layout"

    wmb = np.zeros((128, NK, REL), np.float32)
    for k in range(NK):
        lo, hi = k * KCH, min((k + 1) * KCH, DIM)
        wmb[:hi - lo, k, :] = relmat[:, lo:hi].T
    wmb = wmb.astype(BF16)

    in_maps, metas = [], []
    for c in range(ncores):
        r0, r1 = int(core_r0[c]), int(core_r1[c])
        rc = r1 - r0
        Xp = np.zeros((Rpad, DPAD), np.float32)
        Xp[:rc, :DIM] = repre[r0:r1]
        xt_part = np.ascontiguousarray(
            Xp.reshape(NJ, 1024, NK, 128).transpose(0, 3, 2, 1)
            .reshape(NJ, 128, NK * 1024)).astype(BF16)

        lab = labels_np[r0:r1]
        O = np.zeros((Rpad, REL), np.float32)
        O[np.arange(rc), lab] = 1.0
        OTfull = np.zeros((128, Rpad), np.float32)
        OTfull[:REL] = O.T
        ot_part = np.ascontiguousarray(
            OTfull.reshape(128, NJ, 1024).transpose(1, 0, 2)).astype(BF16)

        blens = lens[c * bpc:(c + 1) * bpc]
        segl = np.repeat(np.arange(bpc, dtype=np.int64), blens)
        seg_pad = np.concatenate(
            [segl, bpc + np.arange(Rpad - rc, dtype=np.int64)])
        chunk_first = seg_pad[(np.arange(Rpad) // 128) * 128]
        seg_local = seg_pad - chunk_first
        assert seg_local.max() <= 127
        H = (seg_local.reshape(NCH, 128)[:, :, None]
             == np.arange(128)[None, None, :])
        h0_part = np.ascontiguousarray(
            H.transpose(1, 0, 2).reshape(128, NCH * 128)
            .reshape(128, NJ, 1024).transpose(1, 0, 2)).astype(FP8)

        xtb = np.concatenate([
            xt_part.view(np.uint8),
            h0_part.view(np.uint8).reshape(NJ, 128, 1024),
            ot_part.view(np.uint8).reshape(NJ, 128, 2048),
        ], axis=2).view(BF16)

        in_maps.append({"xtb": xtb, "wmb": wmb})

        ls = starts[c * bpc:(c + 1) * bpc] - r0
        le = ends[c * bpc:(c + 1) * bpc] - r0
        k0 = ls // 128
        k1 = (le - 1) // 128
        bidx = np.arange(bpc, dtype=np.int64)
        slot0 = bidx - chunk_first[k0 * 128]
        slot1 = bidx - chunk_first[k1 * 128]
        assert slot0.min() >= 0 and slot0.max() <= 127
        assert slot1.min() >= 0 and slot1.max() <= 127
        metas.append((k0, slot0, k1, slot1))
    return in_maps, metas, bias_np, Rpad, bpc


def _compact(results, metas, bias_np, bpc, Rpad):
    NCH = Rpad // 128
    out = np.empty((len(results) * bpc, REL), np.float32)
    for c, res in enumerate(results):
        stage = np.asarray(res["attstage"]).astype(np.float32) \
            .reshape(128, NCH, AUG)
        k0, slot0, k1, slot1 = metas[c]
        acc = stage[slot0, k0, :].copy()
        two = k1 > k0
        acc[two] += stage[slot1[two], k1[two], :]
        out[c * bpc:(c + 1) * bpc] = acc[:, :REL] / acc[:, REL:AUG]
    out += bias_np[None, :]
    return out


def kernel(repre, relation_mat, bias, scope, labels):
    global LAST_RESULTS
    from concourse.bass_utils import run_bass_kernel_spmd

    in_maps, metas, bias_np, Rpad, bpc = _prep(
        repre, relation_mat, bias, scope, labels, NCORES)
    if Rpad not in _PROGRAM_CACHE:
        _PROGRAM_CACHE[Rpad] = _build_program(Rpad)
    nc = _PROGRAM_CACHE[Rpad]
    res = run_bass_kernel_spmd(nc, in_maps, core_ids=list(range(NCORES)),
                               trace=bool(os.environ.get("BASS_TRACE")))
    LAST_RESULTS = res
    return _compact(res.results, metas, bias_np, bpc, Rpad)


# revision 51
# speedup vs baseline: 1.1997x; 1.1997x over previous
"""Trainium2 Bass kernel for nn_AttentionSelector (segment softmax attention).

Math shortcut: logits = segment_sum(w * repre) @ relation_mat.T + bias is
linear in repre, so with P = repre @ relation_mat.T ([N,53]) the whole
computation lives in 53-dim space:
    x_i   = P[i, labels[i]]          (rel logit per instance)
    e_i   = exp(x_i)                 (logits are ~N(0, 0.026^2): no max needed)
    out_b = (sum_{i in b} e_i P[i,:]) / (sum_{i in b} e_i) + bias

Device pipeline (per core, bags sharded 3125/core, rows padded to Rpad):
  A:  stream X^T in bf16 (the HBM roofline) as [128, 6*1024]-blocks
      (128-partition, 12KB contiguous lines -> ~340GB/s vs 135 for 2KB
      packets); 6 accumulating matmuls (D padded 690->768=6*128) per
      512-col half -> P^T in PSUM; scalar-engine copy to bf16 SBUF.
  T:  PE-transposes P^T -> row-major 128-row chunks (4 chunks per PSUM
      tile); per chunk: fused DVE multiply-reduce against a host-built
      one-hot label mask extracts x; scalar engine exp writes e directly
      into the P_aug e-column; pool engine scales P by e into P_aug and
      builds the one-hot slot matrix H0 from host-built seg ids.
  C:  per chunk one matmul att_un[s,0:54] = sum_i H0[i,s]*[eP | e][i,:]
      accumulated 9 chunks per PSUM bank; DVE copies groups to SBUF.
  Host compacts the <=2 (chunk,slot) partials per bag, divides by den,
  adds bias. All DMAs are large contiguous-per-partition transfers
  (no 4-byte scatter packets anywhere).
"""
import math
import os
import sys

for _p in ("/opt/trn_rl_repo", "/opt/trn_rl_repo/concourse", "/opt/pypackages"):
    if _p not in sys.path:
        sys.path.insert(0, _p)

import numpy as np
import ml_dtypes

BF16 = ml_dtypes.bfloat16
FP8 = ml_dtypes.float8_e4m3fn

N_TOTAL = 200000
NUM_BAGS = 25000
DIM = 690
DPAD = 768         # 6 * 128
KCH = 128
NK = 6
REL = 53
AUG = REL + 1      # 53 P-columns + e column
GRP = 9            # attention chunks accumulated per PSUM bank (9*54=486 f32)
NCORES = 8

LAST_RESULTS = None
_PROGRAM_CACHE = {}


def _build_program(Rpad, debug_out=False, stages="ATC", scalar_dma=True):
    from concourse import bacc, mybir
    import concourse.tile as tile
    from concourse.masks import make_identity

    f32 = mybir.dt.float32
    bf16 = mybir.dt.bfloat16
    fp8 = mybir.dt.float8e4
    Alu = mybir.AluOpType
    Act = mybir.ActivationFunctionType
    NJ = Rpad // 1024
    NCH = Rpad // 128
    T = 2 * NJ                      # number of 512-col halves

    nc = bacc.Bacc("TRN2", target_bir_lowering=False, debug=False,
                   enable_asserts=False)

    with tile.TileContext(nc) as tc:
        with tc.tile_pool(name="dram", bufs=1, space="DRAM") as dram, \
             tc.tile_pool(name="consts", bufs=1) as consts, \
             tc.tile_pool(name="xt", bufs=4) as xtp, \
             tc.tile_pool(name="junk", bufs=3) as junkp, \
             tc.tile_pool(name="pte", bufs=3) as ptep, \
             tc.tile_pool(name="erow", bufs=3) as erp, \
             tc.tile_pool(name="big", bufs=1) as bigp, \
             tc.tile_pool(name="pt_ps", bufs=2, space="PSUM") as ptps, \
             tc.tile_pool(name="tr_ps", bufs=2, space="PSUM") as trps, \
             tc.tile_pool(name="x_ps", bufs=2, space="PSUM") as xps, \
             tc.tile_pool(name="c_ps", bufs=2, space="PSUM") as cps:

            # merged per-block stream: [xt bf16 | h0 fp8 | ot fp8] with
            # uniform 14336-byte partition lines (one DMA per block)
            BCOLS = NK * 1024 + 512 + 1024      # in bf16 columns
            H0OFF = NK * 1024                   # h0: 512 bf16 cols = 1024 fp8
            OTOFF = NK * 1024 + 512             # ot: 1024 bf16 cols
            xt_d = dram.tile([NJ, 128, BCOLS], bf16, kind="ExternalInput",
                             name="xtb", uniquify=False)
            wm_d = dram.tile([128, NK, REL], bf16, kind="ExternalInput",
                             name="wmb", uniquify=False)
            att_d = dram.tile([128, NCH * AUG], bf16, kind="ExternalOutput",
                              name="attstage", uniquify=False)
            # constants
            wm_sb = consts.tile([128, NK, REL], bf16, name="wm_sb", tag="wm_sb")
            nc.sync.dma_start(wm_sb[:], wm_d[:])
            identb = consts.tile([128, 128], bf16, name="identb", tag="identb")
            make_identity(nc, identb[:])
            onesb = consts.tile([REL, AUG], bf16, name="onesb", tag="onesb")
            nc.vector.memset(onesb[:], 1.0)

            P_all = bigp.tile([128, NCH * AUG], bf16, name="P_all",
                              tag="P_all")
            attst = bigp.tile([128, NCH * AUG], bf16, name="attst",
                              tag="attst")
            if "C" not in stages:
                nc.vector.memset(attst[:], 0.0)
            # static P^T staging tiles with a permanent ones-row: transposing
            # [54,128] blocks yields [P | 1] slots, so den comes for free
            pt_sbs = []
            for i in range(3):
                t_ = consts.tile([AUG, 512], bf16, name=f"pt_sb{i}",
                                 tag=f"pt_sb{i}")
                nc.vector.memset(t_[:], 1.0)
                pt_sbs.append(t_)

            xt_tiles = {}
            pt_tiles = {}
            junk_tiles = {}
            pte_tiles = {}
            tr_tiles = {}
            cgrp = {"tile": None, "base": 0, "cnt": 0}

            out_state = {"done": 0}

            def flush_cgrp(final=False):
                t_, base, cnt = cgrp["tile"], cgrp["base"], cgrp["cnt"]
                if t_ is not None and cnt > 0:
                    nc.vector.tensor_copy(
                        attst[:, AUG * base:AUG * (base + cnt)],
                        t_[:, :AUG * cnt])
                    cgrp["tile"] = None
                    cgrp["cnt"] = 0
                # stream finished attst ranges out instead of one tail DMA
                hi = base + cnt if t_ is not None else out_state["done"]
                if final:
                    hi = NCH
                if hi - out_state["done"] >= 45 or \
                        (final and hi > out_state["done"]):
                    lo = out_state["done"]
                    nc.scalar.dma_start(att_d[:, AUG * lo:AUG * hi],
                                      attst[:, AUG * lo:AUG * hi])
                    out_state["done"] = hi

            for t in range(T + 3):
                # ---- stage A: matmuls for half t ----
                if t < T:
                    j, h = divmod(t, 2)
                    if h == 0:
                        xt = xtp.tile([128, BCOLS], bf16, name="xt",
                                      tag="xt")
                        nc.sync.dma_start(xt[:], xt_d[j])
                        xt_tiles[j] = xt
                    xt = xt_tiles[j]
                    pt_ps = ptps.tile([REL, 512], f32, space="PSUM",
                                      name="pt_ps", tag="pt_ps")
                    for k in range(NK):
                        nc.tensor.matmul(
                            pt_ps[:], wm_sb[:, k, :],
                            xt[:, 1024 * k + 512 * h:1024 * k + 512 * (h + 1)],
                            start=(k == 0), stop=(k == NK - 1))
                    pt_sb = pt_sbs[t % 3]
                    nc.scalar.activation(pt_sb[:REL, :], pt_ps[:], Act.Copy)
                    pt_tiles[t] = pt_sb
                    junk = junkp.tile([REL, 512], bf16, name="junk",
                                      tag="junk")
                    nc.vector.tensor_tensor(
                        out=junk[:], in0=pt_sb[:REL, :],
                        in1=xt[:REL, OTOFF + 512 * h:OTOFF + 512 * (h + 1)],
                        op=Alu.mult)
                    junk_tiles[t] = junk

                # ---- stage X: x^T, e^T, column-scale for half t-1 ----
                u1 = t - 1
                if "T" in stages and 0 <= u1 < T:
                    junk = junk_tiles.pop(u1)
                    xT_ps = xps.tile([AUG, 512], f32, space="PSUM",
                                     name="xT", tag="xT")
                    nc.tensor.matmul(xT_ps[:], onesb[:], junk[:],
                                     start=True, stop=True)
                    e_bc = erp.tile([AUG, 512], bf16, name="erow", tag="erow")
                    nc.scalar.activation(e_bc[:], xT_ps[:], Act.Exp)
                    pt_e = ptep.tile([AUG, 512], bf16, name="pte", tag="pte")
                    nc.vector.tensor_tensor(
                        out=pt_e[:], in0=pt_tiles.pop(u1),
                        in1=e_bc[:], op=Alu.mult)
                    pte_tiles[u1] = pt_e

                # ---- stage T: transposes + P_all copy for half t-2 ----
                u = t - 2
                if "T" in stages and 0 <= u < T:
                    pt_e = pte_tiles.pop(u)
                    c0 = 4 * u
                    tr4 = trps.tile([128, 4 * AUG], bf16, space="PSUM",
                                    name="tr4", tag="tr4")
                    for q in range(4):
                        nc.tensor.transpose(
                            tr4[:, AUG * q:AUG * (q + 1)],
                            pt_e[:, 128 * q:128 * (q + 1)],
                            identb[:AUG, :AUG])
                    tr_tiles[u] = tr4
                    nc.scalar.activation(
                        P_all[:, AUG * c0:AUG * (c0 + 4)], tr4[:], Act.Copy)

                # ---- stage C: attention matmuls for half t-3 ----
                v = t - 3
                if "C" in stages and 0 <= v < T:
                    tr_tiles.pop(v, None)
                    for q in range(4):
                        c = 4 * v + q
                        if cgrp["tile"] is None:
                            cgrp["tile"] = cps.tile(
                                [128, GRP * AUG], f32, space="PSUM",
                                name="cacc", tag="cacc")
                            cgrp["base"] = c
                        off = AUG * (c - cgrp["base"])
                        cj, cq = divmod(c, 8)
                        h0v = xt_tiles[cj][:, H0OFF + 64 * cq:
                                           H0OFF + 64 * (cq + 1)].bitcast(fp8)
                        nc.tensor.matmul(
                            cgrp["tile"][:, off:off + AUG], h0v,
                            P_all[:, AUG * c:AUG * (c + 1)],
                            start=True, stop=True)
                        cgrp["cnt"] += 1
                        if cgrp["cnt"] == GRP:
                            flush_cgrp()
            flush_cgrp(final=True)

    nc.compile()
    return nc


def _prep(repre, relation_mat, bias, scope, labels, ncores):
    repre = np.asarray(repre, dtype=np.float32)
    relmat = np.asarray(relation_mat, dtype=np.float32)
    bias_np = np.asarray(bias, dtype=np.float32)
    scope = np.asarray(scope).astype(np.int64)
    labels_np = np.asarray(labels).astype(np.int64)
    n, d = repre.shape
    nbags = scope.shape[0]
    assert d == DIM and nbags % ncores == 0
    bpc = nbags // ncores
    starts, ends = scope[:, 0], scope[:, 1]
    lens = ends - starts
    core_r0 = starts[np.arange(ncores) * bpc]
    core_r1 = ends[np.arange(ncores) * bpc + bpc - 1]
    rows = core_r1 - core_r0
    Rpad = int(1024 * math.ceil(int(rows.max()) / 1024))
    NCH = Rpad // 128
    NJ = Rpad // 1024
    assert int(lens.max()) <= 128, "bag too large for this kernel layout"

    wmb = np.zeros((128, NK, REL), np.float32)
    for k in range(NK):
        lo, hi = k * KCH, min((k + 1) * KCH, DIM)
        wmb[:hi - lo, k, :] = relmat[:, lo:hi].T
    wmb = wmb.astype(BF16)

    in_maps, metas = [], []
    for c in range(ncores):
        r0, r1 = int(core_r0[c]), int(core_r1[c])
        rc = r1 - r0
        Xp = np.zeros((Rpad, DPAD), np.float32)
        Xp[:rc, :DIM] = repre[r0:r1]
        xt_part = np.ascontiguousarray(
            Xp.reshape(NJ, 1024, NK, 128).transpose(0, 3, 2, 1)
            .reshape(NJ, 128, NK * 1024)).astype(BF16)

        lab = labels_np[r0:r1]
        O = np.zeros((Rpad, REL), np.float32)
        O[np.arange(rc), lab] = 1.0
        OTfull = np.zeros((128, Rpad), np.float32)
        OTfull[:REL] = O.T
        ot_part = np.ascontiguousarray(
            OTfull.reshape(128, NJ, 1024).transpose(1, 0, 2)).astype(BF16)

        blens = lens[c * bpc:(c + 1) * bpc]
        segl = np.repeat(np.arange(bpc, dtype=np.int64), blens)
        seg_pad = np.concatenate(
            [segl, bpc + np.arange(Rpad - rc, dtype=np.int64)])
        chunk_first = seg_pad[(np.arange(Rpad) // 128) * 128]
        seg_local = seg_pad - chunk_first
        assert seg_local.max() <= 127
        H = (seg_local.reshape(NCH, 128)[:, :, None]
             == np.arange(128)[None, None, :])
        h0_part = np.ascontiguousarray(
            H.transpose(1, 0, 2).reshape(128, NCH * 128)
            .reshape(128, NJ, 1024).transpose(1, 0, 2)).astype(FP8)

        xtb = np.concatenate([
            xt_part.view(np.uint8),
            h0_part.view(np.uint8).reshape(NJ, 128, 1024),
            ot_part.view(np.uint8).reshape(NJ, 128, 2048),
        ], axis=2).view(BF16)

        in_maps.append({"xtb": xtb, "wmb": wmb})

        ls = starts[c * bpc:(c + 1) * bpc] - r0
        le = ends[c * bpc:(c + 1) * bpc] - r0
        k0 = ls // 128
        k1 = (le - 1) // 128
        bidx = np.arange(bpc, dtype=np.int64)
        slot0 = bidx - chunk_first[k0 * 128]
        slot1 = bidx - chunk_first[k1 * 128]
        assert slot0.min() >= 0 and slot0.max() <= 127
        assert slot1.min() >= 0 and slot1.max() <= 127
        metas.append((k0, slot0, k1, slot1))
    return in_maps, metas, bias_np, Rpad, bpc


def _compact(results, metas, bias_np, bpc, Rpad):
    NCH = Rpad // 128
    out = np.empty((len(results) * bpc, REL), np.float32)
    for c, res in enumerate(results):
        stage = np.asarray(res["attstage"]).astype(np.float32) \
            .reshape(128, NCH, AUG)
        k0, slot0, k1, slot1 = metas[c]
        acc = stage[slot0, k0, :].copy()
        two = k1 > k0
        acc[two] += stage[slot1[two], k1[two], :]
        out[c * bpc:(c + 1) * bpc] = acc[:, :REL] / acc[:, REL:AUG]
    out += bias_np[None, :]
    return out


def kernel(repre, relation_mat, bias, scope, labels):
    global LAST_RESULTS
    from concourse.bass_utils import run_bass_kernel_spmd

    in_maps, metas, bias_np, Rpad, bpc = _prep(
        repre, relation_mat, bias, scope, labels, NCORES)
    if Rpad not in _PROGRAM_CACHE:
        _PROGRAM_CACHE[Rpad] = _build_program(Rpad)
    nc = _PROGRAM_CACHE[Rpad]
    res = run_bass_kernel_spmd(nc, in_maps, core_ids=list(range(NCORES)),
                               trace=bool(os.environ.get("BASS_TRACE")))
    LAST_RESULTS = res
    return _compact(res.results, metas, bias_np, bpc, Rpad)
